# revision 3
# baseline (speedup 1.0000x reference)
"""Multi-head causal attention (B=2, T=2048, C=1024, H=16, HS=64) on 8 TRN2
NeuronCores.

Sharding: batch x head-group. Core c handles batch c//4 and heads
[4*(c%4), 4*(c%4)+4), organized as 2 head-pairs. Each core computes a partial
output [T, C] for its batch (row-shard of w_proj over its 256 contraction
columns); the host sums 4 partials per batch and adds b_proj.

Per-core kernel (matmuls in float32r; P/V in bf16):
  - Inputs are host-packed so every DMA is one contiguous run per partition
    (16KB for xT tiles, 8KB for weights), spread across 4 DMA queues so
    compute starts ~4us in.
  - QT/KT/VT [128(2 heads x 64), T] per pair via lhsT=weight chunks,
    rhs=xT chunks.
  - V_aug[h] [keys, 128]: V (cols 0:64 via pair-level PE transpose of VT) |
    ones (64:128, memset).  The ones columns make the O^T matmul produce the
    softmax denominator l in rows 64:128 for free.
  - Flash-style causal attention in transposed layout: S^T[keys, q] blocks
    via lhsT=KT block, rhs=QT slice; ONE exp ACT per [128, 2, 512] PSUM pair
    (diagonal pairs exp above-diagonal garbage which the tri-mask multiply
    and O-matmul column trim exclude); O^T accumulated over key blocks.
  - Normalize with reciprocal_approx_fast; project with lhsT=Ohat t-chunks
    accumulated over the two pairs, rhs=w_projT slice.
  - QKV for the next t-group and proj for the previous group are woven
    between attention jg-steps so the in-order PE queue always has
    independent work behind the exp-dependent O matmuls.
"""

import math
import sys
from collections import deque
from contextlib import ExitStack

if "/opt/trn_rl_repo" not in sys.path:
    sys.path.insert(0, "/opt/trn_rl_repo")

import numpy as np

import concourse.mybir as mybir
import concourse.tile as tile
from concourse import bacc
from concourse.bass import ts
from concourse.bass_utils import run_bass_kernel_spmd

B, T, C = 2, 2048, 1024
H, HS = 16, 64
NCORES = 8
P = 128
G = 512  # q-group size
NG = T // G
KB = 128  # key block
NPO = C // P  # contraction chunks
F32 = mybir.dt.float32
F32R = mybir.dt.float32r
BF16 = mybir.dt.bfloat16

_nc_cache = {}


def _emit(tc):
    nc = tc.nc
    xt4 = nc.dram_tensor("xt4", [NG, P, NPO, G], F32R, kind="ExternalInput").ap()
    w3 = nc.dram_tensor("w3", [3, P, 2, NPO, 128], F32R, kind="ExternalInput").ap()
    wpt = nc.dram_tensor("wpt", [P, 2, C], F32R, kind="ExternalInput").ap()
    trid = nc.dram_tensor("tri", [P, P], BF16, kind="ExternalInput").ap()
    identd = nc.dram_tensor("ident", [P, P], BF16, kind="ExternalInput").ap()
    out = nc.dram_tensor("out", [T, C], F32, kind="ExternalOutput").ap()

    ctx = ExitStack()
    persist = ctx.enter_context(tc.tile_pool(name="persist", bufs=1))
    vt_pool = ctx.enter_context(tc.tile_pool(name="vtp", bufs=2))
    pt_pool = ctx.enter_context(tc.tile_pool(name="ptp", bufs=4))
    norm_pool = ctx.enter_context(tc.tile_pool(name="normp", bufs=2))
    out_pool = ctx.enter_context(tc.tile_pool(name="outp", bufs=2))
    st_psum = ctx.enter_context(tc.tile_pool(name="stps", bufs=2, space="PSUM"))
    ot_psum = ctx.enter_context(tc.tile_pool(name="otps", bufs=2, space="PSUM"))
    mm_psum = ctx.enter_context(tc.tile_pool(name="mmps", bufs=2, space="PSUM"))

    wq_sb = persist.tile([P, 2, NPO, 128], F32R, tag="wq")
    wk_sb = persist.tile([P, 2, NPO, 128], F32R, tag="wk")
    wv_sb = persist.tile([P, 2, NPO, 128], F32R, tag="wv")
    wpt_sb = persist.tile([P, 2, C], F32R, tag="wpt")
    tri_sb = persist.tile([P, P], BF16, tag="tri")
    ident = persist.tile([P, P], BF16, tag="ident")
    xts = [persist.tile([P, NPO, G], F32R, tag=f"xt{tg}", name=f"xt{tg}")
           for tg in range(NG)]
    qt = [persist.tile([P, T], F32R, tag=f"qt{p}", name=f"qt{p}") for p in range(2)]
    kt = [persist.tile([P, T], F32R, tag=f"kt{p}", name=f"kt{p}") for p in range(2)]
    ohat = [persist.tile([P, T], F32R, tag=f"oh{p}", name=f"oh{p}") for p in range(2)]
    # per-head V|ones; heads 2*p+hh live in vaug[2*p+hh]
    vaug = [persist.tile([P, T // KB, 128], BF16, tag=f"va{h}", name=f"va{h}")
            for h in range(4)]

    # ---- input loading: one contiguous run per partition, 4 queues ----
    nc.gpsimd.dma_start(xts[0][:], xt4[0])
    nc.sync.dma_start(wq_sb[:], w3[0])
    nc.scalar.dma_start(wv_sb[:], w3[2])
    nc.sync.dma_start(wk_sb[:], w3[1])
    nc.scalar.dma_start(tri_sb[:], trid[:])
    nc.scalar.dma_start(ident[:], identd[:])
    nc.sync.dma_start(xts[1][:], xt4[1])
    nc.gpsimd.dma_start(xts[2][:], xt4[2])
    nc.scalar.dma_start(xts[3][:], xt4[3])
    nc.sync.dma_start(wpt_sb[:], wpt[:])
    for h in range(4):
        nc.gpsimd.memset(vaug[h][:, :, 64:128], 1.0)

    def emit_qkv(p, which, tg):
        w_sb = {"q": wq_sb, "k": wk_sb, "v": wv_sb}[which]
        ps = mm_psum.tile([P, G], F32, tag="mm", name=f"qkv{p}{which}{tg}")
        for po in range(NPO):
            nc.tensor.matmul(
                ps[:],
                w_sb[:, p, po, :],
                xts[tg][:, po, :],
                start=(po == 0),
                stop=(po == NPO - 1),
            )
        if which == "q":
            nc.vector.tensor_copy(qt[p][:, ts(tg, G)], ps[:])
        elif which == "k":
            nc.vector.tensor_copy(kt[p][:, ts(tg, G)], ps[:])
        else:
            vt = vt_pool.tile([P, G], BF16, tag="vt", name=f"vt{p}{tg}")
            nc.vector.tensor_copy(vt[:], ps[:])
            for kk in range(4):
                j = 4 * tg + kk
                trp = mm_psum.tile([P, P], BF16, tag="mm", name=f"tr{p}{j}")
                nc.tensor.transpose(trp[:], vt[:, ts(kk, P)], ident[:])
                nc.vector.tensor_copy(vaug[2 * p][:, j, 0:64], trp[:, 0:64])
                nc.vector.tensor_copy(vaug[2 * p + 1][:, j, 0:64], trp[:, 64:128])

    def emit_attn(p, g, fillers):
        qtp, ktp, ohp = qt[p], kt[p], ohat[p]
        l_sb = norm_pool.tile([P, G], F32, tag="lsb", name=f"l{p}{g}")
        rinv = norm_pool.tile([P, G], F32, tag="rinv", name=f"r{p}{g}")
        otps_h = [
            ot_psum.tile([P, G], F32, tag="ot", name=f"ot{p}{g}{h}") for h in range(2)
        ]
        n_j = 4 * g + 4
        for jg in range(math.ceil(n_j / 2)):
            js = [j for j in (2 * jg, 2 * jg + 1) if j < n_j]
            stps_h = [
                st_psum.tile([P, 2, G], F32, tag="st", name=f"st{h}")
                for h in range(2)
            ]
            pt_h = [
                pt_pool.tile([P, 2, G], BF16, tag="pt", name=f"pt{h}")
                for h in range(2)
            ]
            for idx, j in enumerate(js):
                r = j - 4 * g
                # g==0: write the full q-range so the PSUM slot is fully
                # initialized before the full-tile exp reads it.
                q0 = 128 * r if (r > 0 and g > 0) else 0
                for h in range(2):
                    hb = 64 * h
                    nc.tensor.matmul(
                        stps_h[h][:, idx, q0:G],
                        ktp[hb : hb + 64, ts(j, KB)],
                        qtp[hb : hb + 64, G * g + q0 : G * (g + 1)],
                        start=True,
                        stop=True,
                    )
            for h in range(2):
                nc.scalar.activation(
                    pt_h[h][:, :, :],
                    stps_h[h][:, :, :],
                    mybir.ActivationFunctionType.Exp,
                    scale=float(HS) ** -0.5,
                )
            for idx, j in enumerate(js):
                r = j - 4 * g
                if r >= 0:
                    q0 = 128 * r
                    for h in range(2):
                        nc.vector.tensor_tensor(
                            pt_h[h][:, idx, q0 : q0 + 128],
                            pt_h[h][:, idx, q0 : q0 + 128],
                            tri_sb[:],
                            mybir.AluOpType.mult,
                        )
            for idx, j in enumerate(js):
                r = j - 4 * g
                q0 = 128 * r if r >= 0 else 0
                for h in range(2):
                    nc.tensor.matmul(
                        otps_h[h][:, q0:G],
                        vaug[2 * p + h][:, j, :],
                        pt_h[h][:, idx, q0:G],
                        start=(j == 0),
                        stop=(j == n_j - 1),
                    )
            if fillers:
                fillers.popleft()()
        stag = norm_pool.tile([P, G], F32, tag="stag", name=f"sg{p}{g}")
        for h in range(2):
            hb = 64 * h
            nc.vector.tensor_copy(stag[hb : hb + 64, :], otps_h[h][0:64, :])
            nc.vector.tensor_copy(l_sb[hb : hb + 64, :], otps_h[h][64:128, :])
        nc.vector.reciprocal_approx_fast(rinv[:], l_sb[:])
        nc.vector.tensor_tensor(
            ohp[:, ts(g, G)], stag[:], rinv[:], mybir.AluOpType.mult
        )

    def proj_chunk(g, tc4):
        t0 = G * g + P * tc4
        o_sb = out_pool.tile([P, C], F32, tag="osb", name=f"osb{g}{tc4}")
        for n in range(C // G):
            pj = mm_psum.tile([P, G], F32, tag="mm", name=f"pj{g}{tc4}{n}")
            for p in range(2):
                nc.tensor.matmul(
                    pj[:],
                    ohat[p][:, t0 : t0 + P],
                    wpt_sb[:, p, ts(n, G)],
                    start=(p == 0),
                    stop=(p == 1),
                )
            nc.vector.tensor_copy(o_sb[:, ts(n, G)], pj[:])
        eng = nc.sync if tc4 % 2 == 0 else nc.gpsimd
        eng.dma_start(out[t0 : t0 + P, :], o_sb[:])

    # ================= emission =================
    for p in range(2):
        for which in ("q", "k", "v"):
            emit_qkv(p, which, 0)
    for g in range(NG):
        fillers = deque()
        if g + 1 < NG:
            for p in range(2):
                for which in ("q", "k", "v"):
                    fillers.append(
                        lambda p=p, w=which, tg=g + 1: emit_qkv(p, w, tg)
                    )
        if g >= 1:
            for tc4 in range(4):
                fillers.append(lambda gg=g - 1, t=tc4: proj_chunk(gg, t))
        emit_attn(0, g, fillers)
        emit_attn(1, g, fillers)
        while fillers:
            fillers.popleft()()
    for tc4 in range(4):
        proj_chunk(NG - 1, tc4)
    ctx.close()


def _build():
    if "nc" in _nc_cache:
        return _nc_cache["nc"]
    nc = bacc.Bacc("TRN2", target_bir_lowering=False, debug=False)
    with tile.TileContext(nc) as tc:
        _emit(tc)
    nc.compile()
    _nc_cache["nc"] = nc
    return nc


def _make_in_maps(x, wq, wk, wv, w_proj):
    import ml_dtypes

    tri = np.triu(np.ones((P, P), dtype=np.float32)).astype(ml_dtypes.bfloat16)
    ident = np.eye(P, dtype=np.float32).astype(ml_dtypes.bfloat16)
    # xt4[b][tg, pi, po, t] = x[b, tg*G + t, po*128 + pi]
    xt4 = [
        np.ascontiguousarray(
            x[b].reshape(NG, G, NPO, P).transpose(0, 3, 2, 1)
        ).astype(np.float32)
        for b in range(B)
    ]

    def pack_w(w, h0):
        # [pi, pair, po, d]: per pair the two heads' [C, 64] slices side by side
        pairs = []
        for pp in range(2):
            wp = np.concatenate([w[h0 + 2 * pp], w[h0 + 2 * pp + 1]], axis=1)
            pairs.append(wp.reshape(NPO, P, P).transpose(1, 0, 2))
        return np.stack(pairs, axis=1)  # [128, 2, NPO, 128]

    in_maps = []
    for c in range(NCORES):
        b, hg = c // 4, c % 4
        h0 = 4 * hg
        w3 = np.ascontiguousarray(
            np.stack([pack_w(wq, h0), pack_w(wk, h0), pack_w(wv, h0)], axis=0)
        ).astype(np.float32)
        # wpt[pi, p, c] = w_proj[c, 256*hg + 128*p + pi]
        wslice = w_proj[:, 256 * hg : 256 * (hg + 1)].T  # [256, C]
        wpt = np.ascontiguousarray(
            wslice.reshape(2, P, C).transpose(1, 0, 2)
        ).astype(np.float32)
        in_maps.append(
            {"xt4": xt4[b], "w3": w3, "wpt": wpt, "tri": tri, "ident": ident}
        )
    return in_maps


def kernel(x, wq, wk, wv, w_proj, b_proj):
    x = np.asarray(x, dtype=np.float32)
    wq = np.asarray(wq, dtype=np.float32)
    wk = np.asarray(wk, dtype=np.float32)
    wv = np.asarray(wv, dtype=np.float32)
    w_proj = np.asarray(w_proj, dtype=np.float32)
    b_proj = np.asarray(b_proj, dtype=np.float32)

    nc = _build()
    in_maps = _make_in_maps(x, wq, wk, wv, w_proj)
    res = run_bass_kernel_spmd(nc, in_maps, core_ids=list(range(NCORES)))
    acc = np.zeros((B, T, C), dtype=np.float64)
    for c, r in enumerate(res.results):
        acc[c // 4] += r["out"]
    return (acc + b_proj).astype(np.float32)


# revision 4
# speedup vs baseline: 1.4418x; 1.4418x over previous
"""Multi-head causal attention (B=2, T=2048, C=1024, H=16, HS=64) on 8 TRN2
NeuronCores.

Sharding: batch x head-group. Core c handles batch c//4 and heads
[4*(c%4), 4*(c%4)+4), organized as 2 head-pairs. Each core computes a partial
output [T, C] for its batch (row-shard of w_proj over its 256 contraction
columns); the host sums 4 partials per batch and adds b_proj.

Per-core kernel (bf16 operands, fp32 PSUM):
  - Inputs are host-packed bf16 so every DMA is one contiguous run per
    partition, spread across 3 DMA queues so compute starts early.
  - QT/KT/VT [128(2 heads x 64), T] per pair via lhsT=weight chunks,
    rhs=xT chunks.
  - V_aug[h] [keys, 128]: V (cols 0:64 via pair-level PE transpose of VT) |
    ones (64:128, memset).  The ones columns make the O^T matmul produce the
    softmax denominator l in rows 64:128 for free.
  - Flash-style causal attention in transposed layout, software-pipelined
    one jg-step deep: step jg emits S^T matmuls + ONE exp ACT over a
    [128, 4, 512] PSUM tile (both heads x two key blocks, trimmed to the
    causal column range on diagonal steps), then the O^T matmuls of step
    jg-1, then one independent filler (QKV for a later t-group / proj for
    an earlier one).  The lag keeps the in-order PE queue free of
    head-of-line stalls behind the scalar-engine exp, and the fillers keep
    the PE HAM clock warm.
  - Normalize with reciprocal_approx_fast; project with lhsT=Ohat t-chunks
    accumulated over the two pairs, rhs=w_projT slice.
"""

import math
import sys
from collections import deque
from contextlib import ExitStack

if "/opt/trn_rl_repo" not in sys.path:
    sys.path.insert(0, "/opt/trn_rl_repo")

import numpy as np

import concourse.mybir as mybir
import concourse.tile as tile
from concourse import bacc
from concourse.bass import ts
from concourse.bass_utils import run_bass_kernel_spmd

B, T, C = 2, 2048, 1024
H, HS = 16, 64
NCORES = 8
P = 128
G = 512  # q-group size
NG = T // G
KB = 128  # key block
NPO = C // P  # contraction chunks
F32 = mybir.dt.float32
BF16 = mybir.dt.bfloat16

_nc_cache = {}


def _emit(tc):
    nc = tc.nc
    xt4 = nc.dram_tensor("xt4", [NG, P, NPO, G], BF16, kind="ExternalInput").ap()
    w3 = nc.dram_tensor("w3", [3, P, 2, NPO, 128], BF16, kind="ExternalInput").ap()
    wpt = nc.dram_tensor("wpt", [P, 2, C], BF16, kind="ExternalInput").ap()
    trid = nc.dram_tensor("tri", [P, P], BF16, kind="ExternalInput").ap()
    identd = nc.dram_tensor("ident", [P, P], BF16, kind="ExternalInput").ap()
    out = nc.dram_tensor("out", [T, C], F32, kind="ExternalOutput").ap()

    ctx = ExitStack()
    persist = ctx.enter_context(tc.tile_pool(name="persist", bufs=1))
    vt_pool = ctx.enter_context(tc.tile_pool(name="vtp", bufs=2))
    pt_pool = ctx.enter_context(tc.tile_pool(name="ptp", bufs=3))
    norm_pool = ctx.enter_context(tc.tile_pool(name="normp", bufs=2))
    out_pool = ctx.enter_context(tc.tile_pool(name="outp", bufs=2))
    st_psum = ctx.enter_context(tc.tile_pool(name="stps", bufs=1, space="PSUM"))
    ot_psum = ctx.enter_context(tc.tile_pool(name="otps", bufs=2, space="PSUM"))
    mm_psum = ctx.enter_context(tc.tile_pool(name="mmps", bufs=2, space="PSUM"))

    wq_sb = persist.tile([P, 2, NPO, 128], BF16, tag="wq")
    wk_sb = persist.tile([P, 2, NPO, 128], BF16, tag="wk")
    wv_sb = persist.tile([P, 2, NPO, 128], BF16, tag="wv")
    wpt_sb = persist.tile([P, 2, C], BF16, tag="wpt")
    tri_sb = persist.tile([P, P], BF16, tag="tri")
    ident = persist.tile([P, P], BF16, tag="ident")
    xts = [persist.tile([P, NPO, G], BF16, tag=f"xt{tg}", name=f"xt{tg}")
           for tg in range(NG)]
    qt = [persist.tile([P, T], BF16, tag=f"qt{p}", name=f"qt{p}") for p in range(2)]
    kt = [persist.tile([P, T], BF16, tag=f"kt{p}", name=f"kt{p}") for p in range(2)]
    ohat = [persist.tile([P, T], BF16, tag=f"oh{p}", name=f"oh{p}") for p in range(2)]
    # per-head V|ones; heads 2*p+hh live in vaug[2*p+hh]
    vaug = [persist.tile([P, T // KB, 128], BF16, tag=f"va{h}", name=f"va{h}")
            for h in range(4)]

    # ---- input loading: one contiguous run per partition, 3 queues ----
    # First-needed first: xt tile 0 split across sync+scalar, then weights.
    nc.sync.dma_start(xts[0][:, 0:4, :], xt4[0][:, 0:4, :])
    nc.scalar.dma_start(xts[0][:, 4:8, :], xt4[0][:, 4:8, :])
    nc.gpsimd.dma_start(wq_sb[:], w3[0])
    nc.sync.dma_start(wk_sb[:], w3[1])
    nc.scalar.dma_start(wv_sb[:], w3[2])
    nc.sync.dma_start(tri_sb[:], trid[:])
    nc.scalar.dma_start(ident[:], identd[:])
    nc.sync.dma_start(xts[1][:], xt4[1])
    nc.gpsimd.dma_start(xts[2][:], xt4[2])
    nc.scalar.dma_start(xts[3][:], xt4[3])
    nc.gpsimd.dma_start(wpt_sb[:], wpt[:])
    for h in range(4):
        nc.gpsimd.memset(vaug[h][:, :, 64:128], 1.0)

    def emit_qkv(p, which, tg):
        w_sb = {"q": wq_sb, "k": wk_sb, "v": wv_sb}[which]
        ps = mm_psum.tile([P, G], F32, tag="mm", name=f"qkv{p}{which}{tg}")
        for po in range(NPO):
            nc.tensor.matmul(
                ps[:],
                w_sb[:, p, po, :],
                xts[tg][:, po, :],
                start=(po == 0),
                stop=(po == NPO - 1),
            )
        if which == "q":
            nc.vector.tensor_copy(qt[p][:, ts(tg, G)], ps[:])
        elif which == "k":
            nc.vector.tensor_copy(kt[p][:, ts(tg, G)], ps[:])
        else:
            vt = vt_pool.tile([P, G], BF16, tag="vt", name=f"vt{p}{tg}")
            nc.vector.tensor_copy(vt[:], ps[:])
            for kk in range(4):
                j = 4 * tg + kk
                trp = mm_psum.tile([P, P], BF16, tag="mm", name=f"tr{p}{j}")
                nc.tensor.transpose(trp[:], vt[:, ts(kk, P)], ident[:])
                nc.vector.tensor_copy(vaug[2 * p][:, j, 0:64], trp[:, 0:64])
                nc.vector.tensor_copy(vaug[2 * p + 1][:, j, 0:64], trp[:, 64:128])

    def emit_attn(p, g, fillers):
        qtp, ktp, ohp = qt[p], kt[p], ohat[p]
        l_sb = norm_pool.tile([P, G], F32, tag="lsb", name=f"l{p}{g}")
        rinv = norm_pool.tile([P, G], F32, tag="rinv", name=f"r{p}{g}")
        otps_h = [
            ot_psum.tile([P, G], F32, tag="ot", name=f"ot{p}{g}{h}") for h in range(2)
        ]
        n_j = 4 * g + 4
        steps = n_j // 2
        prev = None
        for jg in range(steps + 1):
            cur = None
            if jg < steps:
                js = (2 * jg, 2 * jg + 1)
                stps = st_psum.tile([P, 4, G], F32, tag="st", name=f"st{p}{g}{jg}")
                ptt = pt_pool.tile([P, 4, G], BF16, tag="pt", name=f"pt{p}{g}{jg}")
                for idx, j in enumerate(js):
                    r = j - 4 * g
                    # g==0 writes the full q-range so the PSUM slot is fully
                    # initialized before any full-tile exp reads it.
                    q0 = 128 * r if (r > 0 and g > 0) else 0
                    for h in range(2):
                        hb = 64 * h
                        nc.tensor.matmul(
                            stps[:, 2 * h + idx, q0:G],
                            ktp[hb : hb + 64, ts(j, KB)],
                            qtp[hb : hb + 64, G * g + q0 : G * (g + 1)],
                            start=True,
                            stop=True,
                        )
                rmin = 2 * jg - 4 * g
                q0m = 128 * rmin if (rmin > 0 and g > 0) else 0
                nc.scalar.activation(
                    ptt[:, :, q0m:G],
                    stps[:, :, q0m:G],
                    mybir.ActivationFunctionType.Exp,
                    scale=float(HS) ** -0.5,
                )
                for idx, j in enumerate(js):
                    r = j - 4 * g
                    if r >= 0:
                        q0 = 128 * r
                        for h in range(2):
                            nc.vector.tensor_tensor(
                                ptt[:, 2 * h + idx, q0 : q0 + 128],
                                ptt[:, 2 * h + idx, q0 : q0 + 128],
                                tri_sb[:],
                                mybir.AluOpType.mult,
                            )
                cur = (js, ptt)
            if fillers:
                fillers.popleft()()
            if prev is not None:
                js_p, pt_p = prev
                for idx, j in enumerate(js_p):
                    r = j - 4 * g
                    q0 = 128 * r if r >= 0 else 0
                    for h in range(2):
                        nc.tensor.matmul(
                            otps_h[h][:, q0:G],
                            vaug[2 * p + h][:, j, :],
                            pt_p[:, 2 * h + idx, q0:G],
                            start=(j == 0),
                            stop=(j == n_j - 1),
                        )
            prev = cur
        stag = norm_pool.tile([P, G], F32, tag="stag", name=f"sg{p}{g}")
        for h in range(2):
            hb = 64 * h
            nc.vector.tensor_copy(stag[hb : hb + 64, :], otps_h[h][0:64, :])
            nc.vector.tensor_copy(l_sb[hb : hb + 64, :], otps_h[h][64:128, :])
        nc.vector.reciprocal_approx_fast(rinv[:], l_sb[:])
        nc.vector.tensor_tensor(
            ohp[:, ts(g, G)], stag[:], rinv[:], mybir.AluOpType.mult
        )

    def proj_chunk(g, tc4):
        t0 = G * g + P * tc4
        o_sb = out_pool.tile([P, C], F32, tag="osb", name=f"osb{g}{tc4}")
        for n in range(C // G):
            pj = mm_psum.tile([P, G], F32, tag="mm", name=f"pj{g}{tc4}{n}")
            for p in range(2):
                nc.tensor.matmul(
                    pj[:],
                    ohat[p][:, t0 : t0 + P],
                    wpt_sb[:, p, ts(n, G)],
                    start=(p == 0),
                    stop=(p == 1),
                )
            nc.vector.tensor_copy(o_sb[:, ts(n, G)], pj[:])
        eng = nc.sync if tc4 % 2 == 0 else nc.gpsimd
        eng.dma_start(out[t0 : t0 + P, :], o_sb[:])

    # ================= emission =================
    for p in range(2):
        for which in ("q", "k", "v"):
            emit_qkv(p, which, 0)
    for g in range(NG):
        fillers = deque()
        if g + 1 < NG:
            for p in range(2):
                for which in ("q", "k", "v"):
                    fillers.append(
                        lambda p=p, w=which, tg=g + 1: emit_qkv(p, w, tg)
                    )
        if g >= 1:
            for tc4 in range(4):
                fillers.append(lambda gg=g - 1, t=tc4: proj_chunk(gg, t))
        emit_attn(0, g, fillers)
        emit_attn(1, g, fillers)
        while fillers:
            fillers.popleft()()
    for tc4 in range(4):
        proj_chunk(NG - 1, tc4)
    ctx.close()


def _build():
    if "nc" in _nc_cache:
        return _nc_cache["nc"]
    nc = bacc.Bacc("TRN2", target_bir_lowering=False, debug=False)
    with tile.TileContext(nc) as tc:
        _emit(tc)
    nc.compile()
    _nc_cache["nc"] = nc
    return nc


def _make_in_maps(x, wq, wk, wv, w_proj):
    import ml_dtypes

    bf = ml_dtypes.bfloat16
    tri = np.triu(np.ones((P, P), dtype=np.float32)).astype(bf)
    ident = np.eye(P, dtype=np.float32).astype(bf)
    # xt4[b][tg, pi, po, t] = x[b, tg*G + t, po*128 + pi]
    xt4 = [
        np.ascontiguousarray(
            x[b].reshape(NG, G, NPO, P).transpose(0, 3, 2, 1)
        ).astype(bf)
        for b in range(B)
    ]

    def pack_w(w, h0):
        # [pi, pair, po, d]: per pair the two heads' [C, 64] slices side by side
        pairs = []
        for pp in range(2):
            wp = np.concatenate([w[h0 + 2 * pp], w[h0 + 2 * pp + 1]], axis=1)
            pairs.append(wp.reshape(NPO, P, P).transpose(1, 0, 2))
        return np.stack(pairs, axis=1)  # [128, 2, NPO, 128]

    in_maps = []
    for c in range(NCORES):
        b, hg = c // 4, c % 4
        h0 = 4 * hg
        w3 = np.ascontiguousarray(
            np.stack([pack_w(wq, h0), pack_w(wk, h0), pack_w(wv, h0)], axis=0)
        ).astype(bf)
        # wpt[pi, p, c] = w_proj[c, 256*hg + 128*p + pi]
        wslice = w_proj[:, 256 * hg : 256 * (hg + 1)].T  # [256, C]
        wpt = np.ascontiguousarray(
            wslice.reshape(2, P, C).transpose(1, 0, 2)
        ).astype(bf)
        in_maps.append(
            {"xt4": xt4[b], "w3": w3, "wpt": wpt, "tri": tri, "ident": ident}
        )
    return in_maps


def kernel(x, wq, wk, wv, w_proj, b_proj):
    x = np.asarray(x, dtype=np.float32)
    wq = np.asarray(wq, dtype=np.float32)
    wk = np.asarray(wk, dtype=np.float32)
    wv = np.asarray(wv, dtype=np.float32)
    w_proj = np.asarray(w_proj, dtype=np.float32)
    b_proj = np.asarray(b_proj, dtype=np.float32)

    nc = _build()
    in_maps = _make_in_maps(x, wq, wk, wv, w_proj)
    res = run_bass_kernel_spmd(nc, in_maps, core_ids=list(range(NCORES)))
    acc = np.zeros((B, T, C), dtype=np.float64)
    for c, r in enumerate(res.results):
        acc[c // 4] += r["out"]
    return (acc + b_proj).astype(np.float32)


# revision 8
# speedup vs baseline: 1.5280x; 1.0598x over previous
"""Multi-head causal attention (B=2, T=2048, C=1024, H=16, HS=64) on 8 TRN2
NeuronCores.

Sharding: batch x head-group. Core c handles batch c//4 and heads
[4*(c%4), 4*(c%4)+4), organized as 2 head-pairs. Each core computes a partial
output [T, C] for its batch (row-shard of w_proj over its 256 contraction
columns); the host sums 4 partials per batch and adds b_proj.

Per-core kernel (bf16 operands, fp32 PSUM):
  - Inputs are host-packed bf16 so every DMA is one contiguous run per
    partition, spread across 3 DMA queues so compute starts early.
  - QT/KT/VT [128(2 heads x 64), T] per pair via lhsT=weight chunks,
    rhs=xT chunks.
  - V_aug[h] [keys, 128]: V (cols 0:64 via pair-level PE transpose of VT,
    4 key blocks per PSUM tile, 2 strided copies) | ones (64:128, memset).
    The ones columns make the O^T matmul produce the softmax denominator l
    in rows 64:128 for free.
  - Flash-style causal attention in transposed layout, software-pipelined
    one jg-step deep: step jg emits S^T matmuls + ONE exp ACT over a
    [128, 4, 512] PSUM tile (both heads x two key blocks, trimmed to the
    causal column range on diagonal steps), then the O^T matmuls of step
    jg-1, then one independent filler popped from a global deque (QKV for
    later t-groups early, proj for earlier groups late, so the final
    attention group - which has no QKV left - still has PE work).  The lag
    keeps the in-order PE queue free of head-of-line stalls behind the
    scalar-engine exp, and the fillers keep the PE HAM clock warm.
  - Normalize with reciprocal_approx_fast; project with lhsT=Ohat t-chunks
    accumulated over the two pairs, rhs=w_projT slice.
"""

import math
import sys
from collections import deque
from contextlib import ExitStack

if "/opt/trn_rl_repo" not in sys.path:
    sys.path.insert(0, "/opt/trn_rl_repo")

import numpy as np

import concourse.mybir as mybir
import concourse.tile as tile
from concourse import bacc
from concourse.bass import ts
from concourse.bass_utils import run_bass_kernel_spmd

B, T, C = 2, 2048, 1024
H, HS = 16, 64
NCORES = 8
P = 128
G = 512  # q-group size
NG = T // G
KB = 128  # key block
NPO = C // P  # contraction chunks
F32 = mybir.dt.float32
BF16 = mybir.dt.bfloat16

_nc_cache = {}


def _emit(tc):
    nc = tc.nc
    xt4 = nc.dram_tensor("xt4", [NG, P, NPO, G], BF16, kind="ExternalInput").ap()
    w3 = nc.dram_tensor("w3", [3, P, 2, NPO, 128], BF16, kind="ExternalInput").ap()
    wpt = nc.dram_tensor("wpt", [P, 2, C], BF16, kind="ExternalInput").ap()
    trid = nc.dram_tensor("tri2", [P, 2, P], BF16, kind="ExternalInput").ap()
    identd = nc.dram_tensor("ident", [P, P], BF16, kind="ExternalInput").ap()
    out = nc.dram_tensor("out", [T, C], F32, kind="ExternalOutput").ap()

    ctx = ExitStack()
    persist = ctx.enter_context(tc.tile_pool(name="persist", bufs=1))
    vt_pool = ctx.enter_context(tc.tile_pool(name="vtp", bufs=2))
    pt_pool = ctx.enter_context(tc.tile_pool(name="ptp", bufs=3))
    norm_pool = ctx.enter_context(tc.tile_pool(name="normp", bufs=2))
    out_pool = ctx.enter_context(tc.tile_pool(name="outp", bufs=2))
    st_psum = ctx.enter_context(tc.tile_pool(name="stps", bufs=1, space="PSUM"))
    ot_psum = ctx.enter_context(tc.tile_pool(name="otps", bufs=2, space="PSUM"))
    mm_psum = ctx.enter_context(tc.tile_pool(name="mmps", bufs=2, space="PSUM"))

    wq_sb = persist.tile([P, 2, NPO, 128], BF16, tag="wq")
    wk_sb = persist.tile([P, 2, NPO, 128], BF16, tag="wk")
    wv_sb = persist.tile([P, 2, NPO, 128], BF16, tag="wv")
    wpt_sb = persist.tile([P, 2, C], BF16, tag="wpt")
    tri_sb = persist.tile([P, 2, P], BF16, tag="tri")
    ident = persist.tile([P, P], BF16, tag="ident")
    xts = [persist.tile([P, NPO, G], BF16, tag=f"xt{tg}", name=f"xt{tg}")
           for tg in range(NG)]
    qt = [persist.tile([P, T], BF16, tag=f"qt{p}", name=f"qt{p}") for p in range(2)]
    kt = [persist.tile([P, T], BF16, tag=f"kt{p}", name=f"kt{p}") for p in range(2)]
    ohat = [persist.tile([P, T], BF16, tag=f"oh{p}", name=f"oh{p}") for p in range(2)]
    # per-head V|ones; heads 2*p+hh live in vaug[2*p+hh]
    vaug = [persist.tile([P, T // KB, 128], BF16, tag=f"va{h}", name=f"va{h}")
            for h in range(4)]

    # ---- input loading: one contiguous run per partition, 3 queues.
    # First compute needs wq + xt0, so those go first on separate queues.
    nc.sync.dma_start(wq_sb[:], w3[0])
    nc.scalar.dma_start(xts[0][:, 4:8, :], xt4[0][:, 4:8, :])
    nc.sync.dma_start(xts[0][:, 0:4, :], xt4[0][:, 0:4, :])
    nc.scalar.dma_start(wv_sb[:], w3[2])
    nc.sync.dma_start(wk_sb[:], w3[1])
    nc.gpsimd.dma_start(xts[2][:], xt4[2])
    nc.scalar.dma_start(ident[:], identd[:])
    nc.sync.dma_start(tri_sb[:], trid[:])
    nc.sync.dma_start(xts[1][:], xt4[1])
    nc.scalar.dma_start(xts[3][:], xt4[3])
    nc.gpsimd.dma_start(wpt_sb[:], wpt[:])
    for h in range(4):
        nc.gpsimd.memset(vaug[h][:, :, 64:128], 1.0)

    def emit_qkv(p, which, tg):
        w_sb = {"q": wq_sb, "k": wk_sb, "v": wv_sb}[which]
        ps = mm_psum.tile([P, G], F32, tag="mm", name=f"qkv{p}{which}{tg}")
        for po in range(NPO):
            nc.tensor.matmul(
                ps[:],
                w_sb[:, p, po, :],
                xts[tg][:, po, :],
                start=(po == 0),
                stop=(po == NPO - 1),
            )
        if which == "q":
            nc.vector.tensor_copy(qt[p][:, ts(tg, G)], ps[:])
        elif which == "k":
            nc.vector.tensor_copy(kt[p][:, ts(tg, G)], ps[:])
        else:
            vt = vt_pool.tile([P, G], BF16, tag="vt", name=f"vt{p}{tg}")
            nc.vector.tensor_copy(vt[:], ps[:])
            trp = mm_psum.tile([P, 4, P], BF16, tag="mm", name=f"tr{p}{tg}")
            for kk in range(4):
                nc.tensor.transpose(trp[:, kk, :], vt[:, ts(kk, P)], ident[:])
            j0 = 4 * tg
            nc.vector.tensor_copy(
                vaug[2 * p][:, j0 : j0 + 4, 0:64], trp[:, :, 0:64]
            )
            nc.vector.tensor_copy(
                vaug[2 * p + 1][:, j0 : j0 + 4, 0:64], trp[:, :, 64:128]
            )

    def emit_attn(p, g, fillers, stride=1):
        qtp, ktp, ohp = qt[p], kt[p], ohat[p]
        l_sb = norm_pool.tile([P, G], F32, tag="lsb", name=f"l{p}{g}")
        rinv = norm_pool.tile([P, G], F32, tag="rinv", name=f"r{p}{g}")
        otps_h = [
            ot_psum.tile([P, G], F32, tag="ot", name=f"ot{p}{g}{h}") for h in range(2)
        ]
        n_j = 4 * g + 4
        steps = n_j // 2
        prev = None
        for jg in range(steps + 1):
            cur = None
            if jg < steps:
                js = (2 * jg, 2 * jg + 1)
                stps = st_psum.tile([P, 4, G], F32, tag="st", name=f"st{p}{g}{jg}")
                ptt = pt_pool.tile([P, 4, G], BF16, tag="pt", name=f"pt{p}{g}{jg}")
                for idx, j in enumerate(js):
                    r = j - 4 * g
                    # g==0 writes the full q-range so the PSUM slot is fully
                    # initialized before any full-tile exp reads it.
                    q0 = 128 * r if (r > 0 and g > 0) else 0
                    for h in range(2):
                        hb = 64 * h
                        nc.tensor.matmul(
                            stps[:, 2 * idx + h, q0:G],
                            ktp[hb : hb + 64, ts(j, KB)],
                            qtp[hb : hb + 64, G * g + q0 : G * (g + 1)],
                            start=True,
                            stop=True,
                        )
                rmin = 2 * jg - 4 * g
                q0m = 128 * rmin if (rmin > 0 and g > 0) else 0
                nc.scalar.activation(
                    ptt[:, :, q0m:G],
                    stps[:, :, q0m:G],
                    mybir.ActivationFunctionType.Exp,
                    scale=float(HS) ** -0.5,
                )
                for idx, j in enumerate(js):
                    r = j - 4 * g
                    if r >= 0:
                        q0 = 128 * r
                        nc.vector.tensor_tensor(
                            ptt[:, 2 * idx : 2 * idx + 2, q0 : q0 + 128],
                            ptt[:, 2 * idx : 2 * idx + 2, q0 : q0 + 128],
                            tri_sb[:],
                            mybir.AluOpType.mult,
                        )
                cur = (js, ptt)
            if fillers and jg % stride == 0:
                fillers.popleft()()
            if prev is not None:
                js_p, pt_p = prev
                for idx, j in enumerate(js_p):
                    r = j - 4 * g
                    q0 = 128 * r if r >= 0 else 0
                    for h in range(2):
                        nc.tensor.matmul(
                            otps_h[h][:, q0:G],
                            vaug[2 * p + h][:, j, :],
                            pt_p[:, 2 * idx + h, q0:G],
                            start=(j == 0),
                            stop=(j == n_j - 1),
                        )
            prev = cur
        stag = norm_pool.tile([P, G], F32, tag="stag", name=f"sg{p}{g}")
        for h in range(2):
            hb = 64 * h
            nc.vector.tensor_copy(stag[hb : hb + 64, :], otps_h[h][0:64, :])
            nc.vector.tensor_copy(l_sb[hb : hb + 64, :], otps_h[h][64:128, :])
        nc.vector.reciprocal_approx_fast(rinv[:], l_sb[:])
        nc.vector.tensor_tensor(
            ohp[:, ts(g, G)], stag[:], rinv[:], mybir.AluOpType.mult
        )

    def proj_chunk(g, tc4):
        t0 = G * g + P * tc4
        o_sb = out_pool.tile([P, C], F32, tag="osb", name=f"osb{g}{tc4}")
        for n in range(C // G):
            pj = mm_psum.tile([P, G], F32, tag="mm", name=f"pj{g}{tc4}{n}")
            for p in range(2):
                nc.tensor.matmul(
                    pj[:],
                    ohat[p][:, t0 : t0 + P],
                    wpt_sb[:, p, ts(n, G)],
                    start=(p == 0),
                    stop=(p == 1),
                )
            nc.vector.tensor_copy(o_sb[:, ts(n, G)], pj[:])
        eng = nc.sync if tc4 % 2 == 0 else nc.gpsimd
        eng.dma_start(out[t0 : t0 + P, :], o_sb[:])

    # ================= emission =================
    # Lead-in: pair 0's QKV for t-group 0 only; everything else is a filler.
    for which in ("q", "k", "v"):
        emit_qkv(0, which, 0)
    fillers = deque()
    for which in ("q", "k", "v"):
        fillers.append(lambda w=which: emit_qkv(1, w, 0))
    for tg in range(1, NG):
        for p in range(2):
            for which in ("q", "k", "v"):
                fillers.append(lambda p=p, w=which, t=tg: emit_qkv(p, w, t))
    for g in range(NG - 1):
        for tc4 in range(4):
            fillers.append(lambda gg=g, t=tc4: proj_chunk(gg, t))
    for g in range(NG):
        stride = 1 if g < 2 else 2
        emit_attn(0, g, fillers, stride)
        emit_attn(1, g, fillers, stride)
    while fillers:
        fillers.popleft()()
    for tc4 in range(4):
        proj_chunk(NG - 1, tc4)
    ctx.close()


def _build():
    if "nc" in _nc_cache:
        return _nc_cache["nc"]
    nc = bacc.Bacc("TRN2", target_bir_lowering=False, debug=False)
    with tile.TileContext(nc) as tc:
        _emit(tc)
    nc.compile()
    _nc_cache["nc"] = nc
    return nc


def _make_in_maps(x, wq, wk, wv, w_proj):
    import ml_dtypes

    bf = ml_dtypes.bfloat16
    tri1 = np.triu(np.ones((P, P), dtype=np.float32))
    tri2 = np.ascontiguousarray(
        np.stack([tri1, tri1], axis=1)
    ).astype(bf)  # [P, 2, P]
    ident = np.eye(P, dtype=np.float32).astype(bf)
    # xt4[b][tg, pi, po, t] = x[b, tg*G + t, po*128 + pi]
    xt4 = [
        np.ascontiguousarray(
            x[b].reshape(NG, G, NPO, P).transpose(0, 3, 2, 1)
        ).astype(bf)
        for b in range(B)
    ]

    def pack_w(w, h0):
        # [pi, pair, po, d]: per pair the two heads' [C, 64] slices side by side
        pairs = []
        for pp in range(2):
            wp = np.concatenate([w[h0 + 2 * pp], w[h0 + 2 * pp + 1]], axis=1)
            pairs.append(wp.reshape(NPO, P, P).transpose(1, 0, 2))
        return np.stack(pairs, axis=1)  # [128, 2, NPO, 128]

    in_maps = []
    for c in range(NCORES):
        b, hg = c // 4, c % 4
        h0 = 4 * hg
        w3 = np.ascontiguousarray(
            np.stack([pack_w(wq, h0), pack_w(wk, h0), pack_w(wv, h0)], axis=0)
        ).astype(bf)
        # wpt[pi, p, c] = w_proj[c, 256*hg + 128*p + pi]
        wslice = w_proj[:, 256 * hg : 256 * (hg + 1)].T  # [256, C]
        wpt = np.ascontiguousarray(
            wslice.reshape(2, P, C).transpose(1, 0, 2)
        ).astype(bf)
        in_maps.append(
            {"xt4": xt4[b], "w3": w3, "wpt": wpt, "tri2": tri2, "ident": ident}
        )
    return in_maps


def kernel(x, wq, wk, wv, w_proj, b_proj):
    x = np.asarray(x, dtype=np.float32)
    wq = np.asarray(wq, dtype=np.float32)
    wk = np.asarray(wk, dtype=np.float32)
    wv = np.asarray(wv, dtype=np.float32)
    w_proj = np.asarray(w_proj, dtype=np.float32)
    b_proj = np.asarray(b_proj, dtype=np.float32)

    nc = _build()
    in_maps = _make_in_maps(x, wq, wk, wv, w_proj)
    res = run_bass_kernel_spmd(nc, in_maps, core_ids=list(range(NCORES)))
    acc = np.zeros((B, T, C), dtype=np.float64)
    for c, r in enumerate(res.results):
        acc[c // 4] += r["out"]
    return (acc + b_proj).astype(np.float32)


# revision 11
# speedup vs baseline: 1.5316x; 1.0024x over previous
"""Multi-head causal attention (B=2, T=2048, C=1024, H=16, HS=64) on 8 TRN2
NeuronCores.

Sharding: batch x head-group. Core c handles batch c//4 and heads
[4*(c%4), 4*(c%4)+4), organized as 2 head-pairs. Each core computes a partial
output [T, C] for its batch (row-shard of w_proj over its 256 contraction
columns); the host sums 4 partials per batch and adds b_proj.

Per-core kernel:
  - All matmul operands are bf16 (fp8 was measured 2-10x over the error
    tolerance); PSUM accumulation is fp32.  The two heads' S^T matmuls
    (contraction K=64 each) are packed into PE row-groups (0,0)/(64,0) via
    tile_position so they run concurrently in the systolic array.
  - Inputs are host-packed so every DMA is one contiguous run per partition,
    ordered by first-need across 3 DMA queues (HBM ~358 GB/s is shared).
  - V_aug[h] [keys, 128]: V (cols 0:64 via pair-level PE transpose of VT,
    4 key blocks per PSUM tile, 2 strided copies) | ones (64:128, memset).
  - Flash-style causal attention in transposed layout, software-pipelined
    one jg-step deep: step jg emits S^T matmuls + ONE exp ACT over a
    [128, 4, 512] PSUM tile (both heads x two key blocks, trimmed to the
    causal column range on diagonal steps), then the O^T matmuls of step
    jg-1, then one independent filler popped from a global deque (QKV for
    later t-groups early, half-proj-chunks for earlier groups late, so the
    final attention group - which has no QKV left - still has PE work).
    The lag keeps the in-order PE queue free of head-of-line stalls behind
    the scalar-engine exp, and the fillers keep the PE HAM clock warm.
  - Normalize with reciprocal_approx_fast; project in bf16 with lhsT=Ohat
    t-chunks accumulated over the two pairs, rhs=w_projT slice.
"""

import math
import sys
from collections import deque
from contextlib import ExitStack

if "/opt/trn_rl_repo" not in sys.path:
    sys.path.insert(0, "/opt/trn_rl_repo")

import numpy as np

import concourse.mybir as mybir
import concourse.tile as tile
from concourse import bacc
from concourse.bass import ts
from concourse.bass_utils import run_bass_kernel_spmd

B, T, C = 2, 2048, 1024
H, HS = 16, 64
NCORES = 8
P = 128
G = 512  # q-group size
NG = T // G
KB = 128  # key block
NPO = C // P  # contraction chunks
F32 = mybir.dt.float32
BF16 = mybir.dt.bfloat16

_nc_cache = {}


def _emit(tc):
    nc = tc.nc
    xt4 = nc.dram_tensor("xt4", [NG, P, NPO, G], BF16, kind="ExternalInput").ap()
    w3 = nc.dram_tensor("w3", [3, P, 2, NPO, 128], BF16, kind="ExternalInput").ap()
    wpt = nc.dram_tensor("wpt", [P, 2, C], BF16, kind="ExternalInput").ap()
    trid = nc.dram_tensor("tri2", [P, 2, P], BF16, kind="ExternalInput").ap()
    identd = nc.dram_tensor("ident", [P, P], BF16, kind="ExternalInput").ap()
    out = nc.dram_tensor("out", [T, C], F32, kind="ExternalOutput").ap()

    ctx = ExitStack()
    persist = ctx.enter_context(tc.tile_pool(name="persist", bufs=1))
    vt_pool = ctx.enter_context(tc.tile_pool(name="vtp", bufs=2))
    pt_pool = ctx.enter_context(tc.tile_pool(name="ptp", bufs=3))
    norm_pool = ctx.enter_context(tc.tile_pool(name="normp", bufs=2))
    out_pool = ctx.enter_context(tc.tile_pool(name="outp", bufs=2))
    st_psum = ctx.enter_context(tc.tile_pool(name="stps", bufs=1, space="PSUM"))
    ot_psum = ctx.enter_context(tc.tile_pool(name="otps", bufs=2, space="PSUM"))
    mm_psum = ctx.enter_context(tc.tile_pool(name="mmps", bufs=2, space="PSUM"))

    wq_sb = persist.tile([P, 2, NPO, 128], BF16, tag="wq")
    wk_sb = persist.tile([P, 2, NPO, 128], BF16, tag="wk")
    wv_sb = persist.tile([P, 2, NPO, 128], BF16, tag="wv")
    wpt_sb = persist.tile([P, 2, C], BF16, tag="wpt")
    tri_sb = persist.tile([P, 2, P], BF16, tag="tri")
    ident = persist.tile([P, P], BF16, tag="ident")
    xts = [persist.tile([P, NPO, G], BF16, tag=f"xt{tg}", name=f"xt{tg}")
           for tg in range(NG)]
    qt = [persist.tile([P, T], BF16, tag=f"qt{p}", name=f"qt{p}") for p in range(2)]
    kt = [persist.tile([P, T], BF16, tag=f"kt{p}", name=f"kt{p}") for p in range(2)]
    ohat = [persist.tile([P, T], BF16, tag=f"oh{p}", name=f"oh{p}") for p in range(2)]
    # per-head V|64s; heads 2*p+hh live in vaug[2*p+hh]
    vaug = [persist.tile([P, T // KB, 128], BF16, tag=f"va{h}", name=f"va{h}")
            for h in range(4)]

    # ---- input loading: one contiguous run per partition, ordered by
    # first-need across 3 queues (they share HBM bandwidth).
    nc.sync.dma_start(xts[0][:, 0:4, :], xt4[0][:, 0:4, :])
    nc.scalar.dma_start(xts[0][:, 4:8, :], xt4[0][:, 4:8, :])
    nc.sync.dma_start(wq_sb[:], w3[0])
    nc.scalar.dma_start(wk_sb[:], w3[1])
    nc.sync.dma_start(wv_sb[:], w3[2])
    nc.scalar.dma_start(ident[:], identd[:])
    nc.sync.dma_start(tri_sb[:], trid[:])
    nc.sync.dma_start(xts[1][:], xt4[1])
    nc.scalar.dma_start(xts[3][:], xt4[3])
    for h in range(4):
        nc.gpsimd.memset(vaug[h][:, :, 64:128], 1.0)
    nc.gpsimd.dma_start(xts[2][:], xt4[2])
    nc.gpsimd.dma_start(wpt_sb[:], wpt[:])

    def emit_qkv(p, which, tg):
        w_sb = {"q": wq_sb, "k": wk_sb, "v": wv_sb}[which]
        ps = mm_psum.tile([P, G], F32, tag="mm", name=f"qkv{p}{which}{tg}")
        for po in range(NPO):
            nc.tensor.matmul(
                ps[:],
                w_sb[:, p, po, :],
                xts[tg][:, po, :],
                start=(po == 0),
                stop=(po == NPO - 1),
            )
        if which == "q":
            nc.vector.tensor_copy(qt[p][:, ts(tg, G)], ps[:])
        elif which == "k":
            nc.vector.tensor_copy(kt[p][:, ts(tg, G)], ps[:])
        else:
            vt = vt_pool.tile([P, G], BF16, tag="vt", name=f"vt{p}{tg}")
            nc.vector.tensor_copy(vt[:], ps[:])
            trp = mm_psum.tile([P, 4, P], BF16, tag="mm", name=f"tr{p}{tg}")
            for kk in range(4):
                nc.tensor.transpose(trp[:, kk, :], vt[:, ts(kk, P)], ident[:])
            j0 = 4 * tg
            nc.vector.tensor_copy(
                vaug[2 * p][:, j0 : j0 + 4, 0:64], trp[:, :, 0:64]
            )
            nc.vector.tensor_copy(
                vaug[2 * p + 1][:, j0 : j0 + 4, 0:64], trp[:, :, 64:128]
            )

    def emit_attn(p, g, fillers):
        qtp, ktp, ohp = qt[p], kt[p], ohat[p]
        l_sb = norm_pool.tile([P, G], F32, tag="lsb", name=f"l{p}{g}")
        rinv = norm_pool.tile([P, G], F32, tag="rinv", name=f"r{p}{g}")
        otps_h = [
            ot_psum.tile([P, G], F32, tag="ot", name=f"ot{p}{g}{h}") for h in range(2)
        ]
        n_j = 4 * g + 4
        steps = n_j // 2
        prev = None
        for jg in range(steps + 1):
            cur = None
            if jg < steps:
                js = (2 * jg, 2 * jg + 1)
                stps = st_psum.tile([P, 4, G], F32, tag="st", name=f"st{p}{g}{jg}")
                ptt = pt_pool.tile([P, 4, G], BF16, tag="pt", name=f"pt{p}{g}{jg}")
                for idx, j in enumerate(js):
                    r = j - 4 * g
                    # g==0 writes the full q-range so the PSUM slot is fully
                    # initialized before any full-tile exp reads it.
                    q0 = 128 * r if (r > 0 and g > 0) else 0
                    for h in range(2):
                        hb = 64 * h
                        nc.tensor.matmul(
                            stps[:, 2 * idx + h, q0:G],
                            ktp[hb : hb + 64, ts(j, KB)],
                            qtp[hb : hb + 64, G * g + q0 : G * (g + 1)],
                            start=True,
                            stop=True,
                            tile_position=(hb, 0),
                        )
                rmin = 2 * jg - 4 * g
                q0m = 128 * rmin if (rmin > 0 and g > 0) else 0
                nc.scalar.activation(
                    ptt[:, :, q0m:G],
                    stps[:, :, q0m:G],
                    mybir.ActivationFunctionType.Exp,
                    scale=float(HS) ** -0.5,
                )
                for idx, j in enumerate(js):
                    r = j - 4 * g
                    if r >= 0:
                        q0 = 128 * r
                        nc.vector.tensor_tensor(
                            ptt[:, 2 * idx : 2 * idx + 2, q0 : q0 + 128],
                            ptt[:, 2 * idx : 2 * idx + 2, q0 : q0 + 128],
                            tri_sb[:],
                            mybir.AluOpType.mult,
                        )
                cur = (js, ptt)
            if fillers:
                fillers.popleft()()
            if prev is not None:
                js_p, pt_p = prev
                for idx, j in enumerate(js_p):
                    r = j - 4 * g
                    q0 = 128 * r if r >= 0 else 0
                    for h in range(2):
                        nc.tensor.matmul(
                            otps_h[h][:, q0:G],
                            vaug[2 * p + h][:, j, :],
                            pt_p[:, 2 * idx + h, q0:G],
                            start=(j == 0),
                            stop=(j == n_j - 1),
                        )
            prev = cur
        stag = norm_pool.tile([P, G], F32, tag="stag", name=f"sg{p}{g}")
        for h in range(2):
            hb = 64 * h
            nc.vector.tensor_copy(stag[hb : hb + 64, :], otps_h[h][0:64, :])
            nc.vector.tensor_copy(l_sb[hb : hb + 64, :], otps_h[h][64:128, :])
        nc.vector.reciprocal_approx_fast(rinv[:], l_sb[:])
        nc.vector.tensor_tensor(
            ohp[:, ts(g, G)], stag[:], rinv[:], mybir.AluOpType.mult
        )

    def make_proj_fillers(g, tc4):
        t0 = G * g + P * tc4
        cell = {}

        def half(n):
            if n == 0:
                cell["o"] = out_pool.tile(
                    [P, C], F32, tag="osb", name=f"osb{g}{tc4}"
                )
            o_sb = cell["o"]
            pj = mm_psum.tile([P, G], F32, tag="mm", name=f"pj{g}{tc4}{n}")
            for p in range(2):
                nc.tensor.matmul(
                    pj[:],
                    ohat[p][:, t0 : t0 + P],
                    wpt_sb[:, p, ts(n, G)],
                    start=(p == 0),
                    stop=(p == 1),
                )
            nc.vector.tensor_copy(o_sb[:, ts(n, G)], pj[:])
            if n == 1:
                eng = nc.sync if tc4 % 2 == 0 else nc.gpsimd
                eng.dma_start(out[t0 : t0 + P, :], o_sb[:])

        return [lambda: half(0), lambda: half(1)]

    # ================= emission =================
    # Lead-in: pair 0's QKV for t-group 0 only; everything else is a filler.
    for which in ("q", "k", "v"):
        emit_qkv(0, which, 0)
    fillers = deque()
    for which in ("q", "k", "v"):
        fillers.append(lambda w=which: emit_qkv(1, w, 0))
    for tg in range(1, NG):
        for p in range(2):
            for which in ("q", "k", "v"):
                fillers.append(lambda p=p, w=which, t=tg: emit_qkv(p, w, t))
    for g in range(NG - 1):
        for tc4 in range(4):
            fillers.extend(make_proj_fillers(g, tc4))
    for g in range(NG):
        emit_attn(0, g, fillers)
        emit_attn(1, g, fillers)
    while fillers:
        fillers.popleft()()
    for tc4 in range(4):
        for f in make_proj_fillers(NG - 1, tc4):
            f()
    ctx.close()


def _build():
    if "nc" in _nc_cache:
        return _nc_cache["nc"]
    nc = bacc.Bacc("TRN2", target_bir_lowering=False, debug=False)
    with tile.TileContext(nc) as tc:
        _emit(tc)
    nc.compile()
    _nc_cache["nc"] = nc
    return nc


def _make_in_maps(x, wq, wk, wv, w_proj):
    import ml_dtypes

    bf = ml_dtypes.bfloat16
    tri1 = np.triu(np.ones((P, P), dtype=np.float32))
    tri2 = np.ascontiguousarray(np.stack([tri1, tri1], axis=1)).astype(bf)
    ident = np.eye(P, dtype=np.float32).astype(bf)
    # xt4[b][tg, pi, po, t] = x[b, tg*G + t, po*128 + pi]
    xt4 = [
        np.ascontiguousarray(
            x[b].reshape(NG, G, NPO, P).transpose(0, 3, 2, 1)
        ).astype(bf)
        for b in range(B)
    ]

    def pack_w(w, h0):
        # [pi, pair, po, d]: per pair the two heads' [C, 64] slices side by side
        pairs = []
        for pp in range(2):
            wp = np.concatenate([w[h0 + 2 * pp], w[h0 + 2 * pp + 1]], axis=1)
            pairs.append(wp.reshape(NPO, P, P).transpose(1, 0, 2))
        return np.stack(pairs, axis=1)  # [128, 2, NPO, 128]

    in_maps = []
    for c in range(NCORES):
        b, hg = c // 4, c % 4
        h0 = 4 * hg
        w3 = np.ascontiguousarray(
            np.stack([pack_w(wq, h0), pack_w(wk, h0), pack_w(wv, h0)], axis=0)
        ).astype(bf)
        # wpt[pi, p, c] = w_proj[c, 256*hg + 128*p + pi]
        wslice = w_proj[:, 256 * hg : 256 * (hg + 1)].T  # [256, C]
        wpt = np.ascontiguousarray(
            wslice.reshape(2, P, C).transpose(1, 0, 2)
        ).astype(bf)
        in_maps.append(
            {"xt4": xt4[b], "w3": w3, "wpt": wpt, "tri2": tri2, "ident": ident}
        )
    return in_maps


def kernel(x, wq, wk, wv, w_proj, b_proj):
    x = np.asarray(x, dtype=np.float32)
    wq = np.asarray(wq, dtype=np.float32)
    wk = np.asarray(wk, dtype=np.float32)
    wv = np.asarray(wv, dtype=np.float32)
    w_proj = np.asarray(w_proj, dtype=np.float32)
    b_proj = np.asarray(b_proj, dtype=np.float32)

    nc = _build()
    in_maps = _make_in_maps(x, wq, wk, wv, w_proj)
    res = run_bass_kernel_spmd(nc, in_maps, core_ids=list(range(NCORES)))
    acc = np.zeros((B, T, C), dtype=np.float64)
    for c, r in enumerate(res.results):
        acc[c // 4] += r["out"]
    return (acc + b_proj).astype(np.float32)
